# revision 43
# baseline (speedup 1.0000x reference)
"""NVFP4 BlackwellLinear kernel for 8 Trainium2 NeuronCores.

Strategy (column-parallel, per sharding hint):
  - weight_q/weight_scale/bias are sharded along out_features (16384 -> 8 x 2048).
  - Weights are prepacked on host: w_deq = weight_q * weight_scale (exact in bf16,
    <= 6 significand bits), shipped pre-transposed as wt[K, N_loc] bf16.
  - x is replicated; each core quantizes the full activation tensor on-device,
    then does the bf16 matmul out^T = w_deq @ x_deq^T with bias fused into the
    PSUM->SBUF eviction. Host transposes/concats the per-core out^T slices.

The entire fp4 round-to-nearest is ONE custom DVE op (QFP4) using a
binade-dependent magic number:
  v2 = x * r2                      (r2 = 2/s per 16-block, broadcast)
  b  = v2 & 0x7F800000             (unsigned binade; AND with +inf imm)
  G  = max(b * M/2, M)             (M = 1.5*2^23; G in {M, 2M, 4M, ...})
  q2 = (v2 + G) - G                == 2*fp4(x/s) exactly for |v2| <= 12.8
Rounding to multiples of 2^t where t = max(0, exp(v2)-2) lands exactly on the
fp4 e2m1 grid {0,1,2,3,4,6,8,12} (x2). Valid while |x/s| <= 7 (guaranteed:
s >= (amax/6)*(15/16) for normal-range scales; blocks with amax < ~0.018
could violate via e4m3-subnormal scale error, never seen for N(0,1) data).
Then xdeq = q2 * (s/2) in bf16 (exact, <= 6 significand bits).
"""

import os
import numpy as np

TOK = 4096
K = 4096
OUT_F = 16384
N_CORES = 8
NL = OUT_F // N_CORES  # 2048
P = 128
BLOCK = 16

# tunables
CHUNK = 512          # max token chunk for the matmul phase (rhs free dim)
CHUNKS = (512,) * 8
QS = 2048            # quant compute slice (free elems) == x DMA granularity
XT_SLOTS = 39        # xT tile slots ([P, CHUNK] bf16 each)
MUL_ON_GPSIMD = False
LDW_OPT = False      # walrus LDW opt is incompatible with bass ldweights
FP8_K = 2048         # leading k-extent computed in fp8 e4m3 DoubleRow (mult of 256)

MAGIC = 12582912.0   # 1.5 * 2^23
FP8_MIN = 2.0 ** -9

_REGISTERED = {}


def _register_ops():
    """Register the custom DVE ops (idempotent). shas computed dynamically."""
    if _REGISTERED:
        return _REGISTERED
    import concourse.dve_ops as dve_ops
    from concourse.dve_ops import DveOp
    from concourse.dve_spec import (
        Spec, Src0, Src1, C0, C1, C2, lower, AluOp, Bin, maxx, _has_src1,
    )
    from concourse.dve_uop import DveOpSpec

    def ref_mulb(in0, in1, s0, s1, imm2):
        a = np.asarray(in0, np.float32)
        b = np.asarray(in1, np.float32).reshape(a.shape)
        return (a * b).astype(np.float32)

    spec_mulb = Spec(body=Src0 * Src1, reference=ref_mulb)

    def ref_round(in0, in1, s0, s1, imm2):
        v2 = np.asarray(in0, np.float32)
        b = (v2.view(np.uint32) & np.uint32(0x7F800000)).view(np.float32)
        G = np.maximum((b * np.float32(s1)).astype(np.float32),
                       np.float32(imm2))
        h = (v2 + G).astype(np.float32)
        return (h - G).astype(np.float32)

    # C0 = +inf mask AP (0x7F800000 per-partition), C1 = MAGIC/2, C2 = MAGIC
    b = Bin(AluOp.BITWISE_AND, Src0, C0)
    G = maxx(b * C1, C2)
    spec_round = Spec(body=(Src0 + G) - G, reference=ref_round)

    def mk(name, spec):
        shas = {}
        for ver in ("v3", "v4"):
            uops = lower(spec, ver=ver)
            row = dve_ops._CUSTOM_DVE_ROW_BASE + len(dve_ops.OPS)
            dos = DveOpSpec(name=name, opcode=row, uops=uops, rd1_en=_has_src1(spec))
            shas[ver] = dos.sha(ver)
        op = DveOp(name, spec, subdim=False, uops_sha=shas)
        dve_ops.OPS.append(op)
        dve_ops.CUSTOM_DVE_SPECS[name] = spec
        dve_ops._SUB_OPCODE_FOR_NAME[name] = dve_ops._CUSTOM_DVE_ROW_BASE + len(dve_ops.OPS) - 1
        return op

    _REGISTERED["MULB"] = mk("NVFP4_MULBCAST_ANT", spec_mulb)
    _REGISTERED["ROUND"] = mk("NVFP4_MAGICROUND_ANT", spec_round)
    return _REGISTERED


_NC_CACHE = {}


_LDW_ENABLE = [False]


def _patch_ldw_opt():
    """Flip walrus --enable-ldw-opt to true (hidden LDWEIGHTS via weight
    preload). The stock command line disables it; measured cost is ~43ns of
    exposed weight-load per 512-col matmul without it."""
    _LDW_ENABLE[0] = True
    from concourse import bass_utils as bu
    if getattr(bu, "_ldw_patched", False):
        return
    orig = bu.run_command

    def run_command_ldw(cmd, *a, **kw):
        if _LDW_ENABLE[0] and isinstance(cmd, list):
            cmd = ["--enable-ldw-opt=true" if c == "--enable-ldw-opt=false"
                   else c for c in cmd]
        return orig(cmd, *a, **kw)

    bu.run_command = run_command_ldw
    bu._ldw_patched = True


def build_nc(tok=TOK, k=K, nl=NL, chunk=CHUNK, qs=QS,
             xt_slots=XT_SLOTS, chunks=None, mul_on_gpsimd=MUL_ON_GPSIMD,
             ldw_opt=LDW_OPT, fp8_k=FP8_K, debug_xdeq=False):
    if ldw_opt:
        _patch_ldw_opt()
    assert fp8_k % 256 == 0 and 0 <= fp8_k < k
    if chunks is None:
        chunks = [c for c in CHUNKS if c <= tok]
        if sum(chunks) != tok:
            chunks = [chunk] * (tok // chunk)
    chunks = tuple(chunks)
    assert sum(chunks) == tok
    key = (tok, k, nl, chunk, qs, xt_slots, chunks, mul_on_gpsimd, ldw_opt,
           fp8_k, debug_xdeq)
    if key in _NC_CACHE:
        return _NC_CACHE[key]

    import concourse.bass as bass
    import concourse.mybir as mybir
    import concourse.tile as tile
    from concourse import bacc

    ops = _register_ops()
    dt = mybir.dt

    KT = k // P            # k-tiles (total)
    K16 = (k - fp8_k) // P  # bf16 k-tiles
    DRB = fp8_k // 256     # fp8 DoubleRow blocks (256 k each)
    NT = nl // P           # n-tiles
    MT = tok // P          # m-tiles (token rows)
    nq = k // qs           # quant slices per m-tile
    nblk = qs // BLOCK     # 16-blocks per quant slice

    nc = bacc.Bacc("TRN2", target_bir_lowering=False, debug=False,
                   num_devices=N_CORES)

    x_d = nc.dram_tensor("x", [tok, k], dt.float32, kind="ExternalInput").ap()
    wt_d = nc.dram_tensor("wt", [k - fp8_k, nl], dt.bfloat16,
                          kind="ExternalInput").ap()
    b_d = nc.dram_tensor("bias", [nl, 1], dt.float32, kind="ExternalInput").ap()
    o_d = nc.dram_tensor("outT", [nl, tok], dt.float32, kind="ExternalOutput").ap()
    xq_d = nc.dram_tensor("xdeq", [tok, k - fp8_k], dt.bfloat16,
                          kind="ExternalOutput" if debug_xdeq else "Internal").ap()
    if fp8_k:
        # fp8 weights, DoubleRow-packed: row-slab b holds w8[p, i, n] with
        # k = b*256 + 2p + i
        w8_d = nc.dram_tensor("w8", [DRB * P, 2 * nl], dt.float8e4,
                              kind="ExternalInput").ap()
        xq8_d = nc.dram_tensor("xdeq8", [tok, fp8_k], dt.float8e4,
                               kind="Internal").ap()

    with tile.TileContext(nc) as tc:
        with (
            tc.tile_pool(name="const", bufs=1) as constp,
            tc.tile_pool(name="wres", bufs=1) as wres,
            tc.tile_pool(name="xin", bufs=2) as xin,
            tc.tile_pool(name="scal", bufs=2) as scal,
            tc.tile_pool(name="q2p", bufs=2) as q2p,
            tc.tile_pool(name="q8p", bufs=2) as q8p,
            tc.tile_pool(name="shp", bufs=2) as shp,
            tc.tile_pool(name="xtp", bufs=xt_slots) as xtp,
            tc.tile_pool(name="outp", bufs=2) as outp,
            tc.tile_pool(name="psum", bufs=4, space="PSUM") as psump,
        ):
            # ---- constants ----
            emask = constp.tile([P, 1], dt.float32, tag="emask")
            nc.vector._memset_packed(emask[:], 0x7F800000)
            bias_t = constp.tile([P, NT], dt.float32, tag="bias")
            for n in range(NT):
                nc.sync.dma_start(bias_t[:, n:n + 1], b_d[n * P:(n + 1) * P, :])

            wt_tiles = []
            w8_tiles = []

            def quant_slice(m, d):
                    xsl = xin.tile([P, qs], dt.float32, tag="xsl")
                    nc.sync.dma_start(
                        xsl[:], x_d[m * P:(m + 1) * P, d * qs:(d + 1) * qs])
                    # per-16-block absmax -> e4m3 scale -> s/2 and 2/s
                    amax = scal.tile([P, nblk], dt.float32, tag="amax")
                    nc.vector.tensor_reduce(
                        amax[:], xsl[:].rearrange("p (b s) -> p b s", s=BLOCK),
                        axis=mybir.AxisListType.X, op=mybir.AluOpType.max,
                        apply_absolute_value=True)
                    s8 = scal.tile([P, nblk], dt.float8e4, tag="s8")
                    nc.vector.tensor_scalar(
                        out=s8[:], in0=amax[:], scalar1=1.0 / 6.0, scalar2=None,
                        op0=mybir.AluOpType.mult)
                    sh = scal.tile([P, nblk], dt.float32, tag="sh")
                    nc.vector.tensor_scalar(
                        out=sh[:], in0=s8[:], scalar1=FP8_MIN, scalar2=0.5,
                        op0=mybir.AluOpType.max, op1=mybir.AluOpType.mult)
                    r2 = scal.tile([P, nblk], dt.float32, tag="r2")
                    rs = scal.tile([P, nblk], dt.float32, tag="rs")
                    nc.vector.reciprocal_approx_accurate(r2[:], sh[:], rs[:])
                    # v2 = x * 2/s (in-place over the x tile), then
                    # q2 = 2*fp4(v2/2) via binade-magic round
                    nc.vector._custom_dve(
                        ops["MULB"], out=xsl[:], in0=xsl[:],
                        in1=r2[:].unsqueeze(2).to_broadcast((P, nblk, BLOCK)))
                    q2 = q2p.tile([P, qs], dt.bfloat16, tag="q2")
                    nc.vector._custom_dve(
                        ops["ROUND"], out=q2[:], in0=xsl[:],
                        s0=emask[:, :], s1=MAGIC / 2, imm2=MAGIC)
                    # s/2 expanded to bf16 (ACT)
                    shx = shp.tile([P, qs], dt.bfloat16, tag="shx")
                    nc.scalar.activation(
                        shx[:].rearrange("p (b s) -> p b s", s=BLOCK),
                        sh[:].unsqueeze(2).to_broadcast((P, nblk, BLOCK)),
                        mybir.ActivationFunctionType.Copy, bias=0.0, scale=1.0)
                    # xdeq = q2 * s/2; leading-fp8_k columns cast to e4m3
                    mul_eng = nc.gpsimd if mul_on_gpsimd else nc.vector
                    k0, k1 = d * qs, (d + 1) * qs
                    n8 = min(max(fp8_k - k0, 0), qs)  # fp8 cols in this slice
                    if n8:
                        q8 = q8p.tile([P, qs], dt.float8e4, tag="q8")
                        mul_eng.tensor_tensor(
                            out=q8[:, :n8], in0=q2[:, :n8], in1=shx[:, :n8],
                            op=mybir.AluOpType.mult)
                        nc.sync.dma_start(
                            xq8_d[m * P:(m + 1) * P, k0:k0 + n8], q8[:, :n8])
                    if n8 < qs:
                        mul_eng.tensor_tensor(
                            out=q2[:, n8:], in0=q2[:, n8:], in1=shx[:, n8:],
                            op=mybir.AluOpType.mult)
                        nc.sync.dma_start(
                            xq_d[m * P:(m + 1) * P,
                                 k0 + n8 - fp8_k:k1 - fp8_k], q2[:, n8:])

            def matmul_chunk(t0, ck):
                # transposed activations: fp8 k-pairs ride as u16 through the
                # xbar transpose; SBUF layout [p, 2t+i] with k = blk*256+2p+i.
                # Transposes alternate between the two HWDGE queues (SP/ACT).
                xt8s = []
                for b in range(DRB):
                    xt8 = xtp.tile([P, chunk], dt.bfloat16, tag="xt", name="xt")[:, :ck]
                    nc.sync.dma_start_transpose(
                        xt8, xq8_d[t0:t0 + ck, b * 256:(b + 1) * 256].bitcast(
                            dt.bfloat16))
                    xt8s.append(xt8)
                xts = []
                for kk in range(K16):
                    xt = xtp.tile([P, chunk], dt.bfloat16, tag="xt", name="xt")[:, :ck]
                    nc.sync.dma_start_transpose(
                        xt, xq_d[t0:t0 + ck, kk * P:(kk + 1) * P])
                    xts.append(xt)
                for n in range(NT):
                    ps = psump.tile([P, chunk], dt.float32, tag="ps", name="ps")[:, :ck]
                    nmm = DRB + K16
                    for b in range(DRB):
                        rhs8 = xt8s[b].bitcast(dt.float8e4).rearrange(
                            "p (t i) -> p i t", i=2)
                        lhs8 = w8_tiles[b][:].rearrange(
                            "p (i n) -> p i n", i=2)[:, :, n * P:(n + 1) * P]
                        nc.tensor.matmul(
                            ps, lhs8, rhs8, start=(b == 0), stop=False,
                            perf_mode=mybir.MatmulPerfMode.DoubleRow,
                            skip_group_check=True)
                    for kk in range(K16):
                        nc.tensor.matmul(
                            ps, wt_tiles[kk][:, n * P:(n + 1) * P], xts[kk],
                            start=(DRB == 0 and kk == 0), stop=(kk == K16 - 1),
                            skip_group_check=bool(DRB))
                    ob = outp.tile([P, chunk], dt.float32, tag="ob", name="ob")[:, :ck]
                    nc.scalar.activation(
                        ob, ps, mybir.ActivationFunctionType.Identity,
                        bias=bias_t[:, n:n + 1], scale=1.0)
                    nc.sync.dma_start(
                        o_d[n * P:(n + 1) * P, t0:t0 + ck], ob)

            # first chunk's quant is issued BEFORE the resident-weight loads so
            # its x DMA + DVE work overlaps the weight fetch; quant runs
            # d-major within each chunk so transposable columns finish early
            t0 = 0
            mdone = 0
            wt_loaded = False
            for ck in chunks:
                t0n = t0 + ck
                m_lo, m_hi = mdone, t0n // P
                for m in range(m_lo, m_hi):
                    for d in range(nq):
                        quant_slice(m, d)
                mdone = m_hi
                if not wt_loaded:
                    for b in range(DRB):
                        t = wres.tile([P, 2 * nl], dt.float8e4, tag=f"w8{b}")
                        nc.sync.dma_start(t[:], w8_d[b * P:(b + 1) * P, :])
                        w8_tiles.append(t)
                    for kk in range(K16):
                        t = wres.tile([P, nl], dt.bfloat16, tag=f"wt{kk}")
                        nc.sync.dma_start(t[:], wt_d[kk * P:(kk + 1) * P, :])
                        wt_tiles.append(t)
                    wt_loaded = True
                matmul_chunk(t0, ck)
                t0 = t0n

    nc.compile()
    _NC_CACHE[key] = nc
    return nc


def _prep_weights(weight_q, weight_scale, bias, fp8_k=FP8_K):
    """Host prepack: per-core transposed dequantized weights.
    Leading fp8_k columns as e4m3 in DoubleRow pair layout (k = b*256+2p+i),
    remainder as plain transposed bf16."""
    import ml_dtypes
    wq = np.asarray(weight_q, np.float32).reshape(OUT_F, K // BLOCK, BLOCK)
    ws = np.asarray(weight_scale, np.float32)[:, :, None]
    wdeq = (wq * ws).reshape(OUT_F, K)  # exact: <=6 significand bits
    wts, w8s, biases = [], [], []
    for c in range(N_CORES):
        sl = wdeq[c * NL:(c + 1) * NL]          # [NL, K]
        wts.append(np.ascontiguousarray(sl[:, fp8_k:].T).astype(
            ml_dtypes.bfloat16))
        if fp8_k:
            w8 = sl[:, :fp8_k].T.astype(ml_dtypes.float8_e4m3fn)  # [fp8_k, NL]
            w8s.append(np.ascontiguousarray(
                w8.reshape(fp8_k // 256, 128, 2, NL).reshape(
                    (fp8_k // 256) * 128, 2 * NL)))
        biases.append(np.ascontiguousarray(
            np.asarray(bias, np.float32)[c * NL:(c + 1) * NL].reshape(NL, 1)))
    return wts, w8s, biases


def make_in_maps(x, weight_q, weight_scale, bias):
    x2 = np.ascontiguousarray(np.asarray(x, np.float32).reshape(TOK, K))
    wts, w8s, biases = _prep_weights(weight_q, weight_scale, bias)
    in_maps = [{"x": x2, "wt": wts[c], "bias": biases[c]} for c in range(N_CORES)]
    if FP8_K:
        for c in range(N_CORES):
            in_maps[c]["w8"] = w8s[c]
    return in_maps


def kernel(x, weight_q, weight_scale, bias):
    from concourse.bass_utils import run_bass_kernel_spmd

    nc = build_nc()
    in_maps = make_in_maps(x, weight_q, weight_scale, bias)
    res = run_bass_kernel_spmd(nc, in_maps, list(range(N_CORES)))
    out = np.empty((TOK, OUT_F), np.float32)
    for c in range(N_CORES):
        out[:, c * NL:(c + 1) * NL] = res.results[c]["outT"].T
    return out.reshape(1, TOK, OUT_F)


if __name__ == "__main__":
    rng = np.random.default_rng(0)
    x = rng.normal(size=(1, TOK, K)).astype(np.float32)
    wq = rng.normal(size=(OUT_F, K)).astype(np.float32)
    ws = rng.random(size=(OUT_F, K // BLOCK)).astype(np.float32) + 0.1
    b = rng.normal(size=(OUT_F,)).astype(np.float32)
    out = kernel(x, wq, ws, b)
    print(out.shape, out.dtype)


# revision 47
# speedup vs baseline: 1.0833x; 1.0833x over previous
"""NVFP4 BlackwellLinear kernel for 8 Trainium2 NeuronCores.

Strategy (column-parallel, per sharding hint):
  - weight_q/weight_scale/bias are sharded along out_features (16384 -> 8 x 2048).
  - Weights are prepacked on host: w_deq = weight_q * weight_scale (exact in bf16,
    <= 6 significand bits), shipped pre-transposed as wt[K, N_loc] bf16.
  - x is replicated; each core quantizes the full activation tensor on-device,
    then does the bf16 matmul out^T = w_deq @ x_deq^T with bias fused into the
    PSUM->SBUF eviction. Host transposes/concats the per-core out^T slices.

The entire fp4 round-to-nearest is ONE custom DVE op (QFP4) using a
binade-dependent magic number:
  v2 = x * r2                      (r2 = 2/s per 16-block, broadcast)
  b  = v2 & 0x7F800000             (unsigned binade; AND with +inf imm)
  G  = max(b * M/2, M)             (M = 1.5*2^23; G in {M, 2M, 4M, ...})
  q2 = (v2 + G) - G                == 2*fp4(x/s) exactly for |v2| <= 12.8
Rounding to multiples of 2^t where t = max(0, exp(v2)-2) lands exactly on the
fp4 e2m1 grid {0,1,2,3,4,6,8,12} (x2). Valid while |x/s| <= 7 (guaranteed:
s >= (amax/6)*(15/16) for normal-range scales; blocks with amax < ~0.018
could violate via e4m3-subnormal scale error, never seen for N(0,1) data).
Then xdeq = q2 * (s/2) in bf16 (exact, <= 6 significand bits).
"""

import os
import numpy as np

TOK = 4096
K = 4096
OUT_F = 16384
N_CORES = 8
NL = OUT_F // N_CORES  # 2048
P = 128
BLOCK = 16

# tunables
CHUNK = 512          # max token chunk for the matmul phase (rhs free dim)
CHUNKS = (512,) * 8
QS = 2048            # quant compute slice (free elems) == x DMA granularity
XT_SLOTS = 44        # xT tile slots ([P, CHUNK] bf16 each)
MUL_ON_GPSIMD = False
LDW_OPT = False      # walrus LDW opt is incompatible with bass ldweights
FP8_K = 2560         # leading k-extent computed in fp8 e4m3 DoubleRow (mult of 256)

MAGIC = 12582912.0   # 1.5 * 2^23
FP8_MIN = 2.0 ** -9

_REGISTERED = {}


def _register_ops():
    """Register the custom DVE ops (idempotent). shas computed dynamically."""
    if _REGISTERED:
        return _REGISTERED
    import concourse.dve_ops as dve_ops
    from concourse.dve_ops import DveOp
    from concourse.dve_spec import (
        Spec, Src0, Src1, C0, C1, C2, lower, AluOp, Bin, maxx, _has_src1,
    )
    from concourse.dve_uop import DveOpSpec

    def ref_mulb(in0, in1, s0, s1, imm2):
        a = np.asarray(in0, np.float32)
        b = np.asarray(in1, np.float32).reshape(a.shape)
        return (a * b).astype(np.float32)

    spec_mulb = Spec(body=Src0 * Src1, reference=ref_mulb)

    def ref_round(in0, in1, s0, s1, imm2):
        v2 = np.asarray(in0, np.float32)
        b = (v2.view(np.uint32) & np.uint32(0x7F800000)).view(np.float32)
        G = np.maximum((b * np.float32(s1)).astype(np.float32),
                       np.float32(imm2))
        h = (v2 + G).astype(np.float32)
        return (h - G).astype(np.float32)

    # C0 = +inf mask AP (0x7F800000 per-partition), C1 = MAGIC/2, C2 = MAGIC
    b = Bin(AluOp.BITWISE_AND, Src0, C0)
    G = maxx(b * C1, C2)
    spec_round = Spec(body=(Src0 + G) - G, reference=ref_round)

    def mk(name, spec):
        shas = {}
        for ver in ("v3", "v4"):
            uops = lower(spec, ver=ver)
            row = dve_ops._CUSTOM_DVE_ROW_BASE + len(dve_ops.OPS)
            dos = DveOpSpec(name=name, opcode=row, uops=uops, rd1_en=_has_src1(spec))
            shas[ver] = dos.sha(ver)
        op = DveOp(name, spec, subdim=False, uops_sha=shas)
        dve_ops.OPS.append(op)
        dve_ops.CUSTOM_DVE_SPECS[name] = spec
        dve_ops._SUB_OPCODE_FOR_NAME[name] = dve_ops._CUSTOM_DVE_ROW_BASE + len(dve_ops.OPS) - 1
        return op

    _REGISTERED["MULB"] = mk("NVFP4_MULBCAST_ANT", spec_mulb)
    _REGISTERED["ROUND"] = mk("NVFP4_MAGICROUND_ANT", spec_round)
    return _REGISTERED


_NC_CACHE = {}


_LDW_ENABLE = [False]


def _patch_ldw_opt():
    """Flip walrus --enable-ldw-opt to true (hidden LDWEIGHTS via weight
    preload). The stock command line disables it; measured cost is ~43ns of
    exposed weight-load per 512-col matmul without it."""
    _LDW_ENABLE[0] = True
    from concourse import bass_utils as bu
    if getattr(bu, "_ldw_patched", False):
        return
    orig = bu.run_command

    def run_command_ldw(cmd, *a, **kw):
        if _LDW_ENABLE[0] and isinstance(cmd, list):
            cmd = ["--enable-ldw-opt=true" if c == "--enable-ldw-opt=false"
                   else c for c in cmd]
        return orig(cmd, *a, **kw)

    bu.run_command = run_command_ldw
    bu._ldw_patched = True


def build_nc(tok=TOK, k=K, nl=NL, chunk=CHUNK, qs=QS,
             xt_slots=XT_SLOTS, chunks=None, mul_on_gpsimd=MUL_ON_GPSIMD,
             ldw_opt=LDW_OPT, fp8_k=FP8_K, debug_xdeq=False):
    if ldw_opt:
        _patch_ldw_opt()
    assert fp8_k % 256 == 0 and 0 <= fp8_k < k
    if chunks is None:
        chunks = [c for c in CHUNKS if c <= tok]
        if sum(chunks) != tok:
            chunks = [chunk] * (tok // chunk)
    chunks = tuple(chunks)
    assert sum(chunks) == tok
    key = (tok, k, nl, chunk, qs, xt_slots, chunks, mul_on_gpsimd, ldw_opt,
           fp8_k, debug_xdeq)
    if key in _NC_CACHE:
        return _NC_CACHE[key]

    import concourse.bass as bass
    import concourse.mybir as mybir
    import concourse.tile as tile
    from concourse import bacc

    ops = _register_ops()
    dt = mybir.dt

    KT = k // P            # k-tiles (total)
    K16 = (k - fp8_k) // P  # bf16 k-tiles
    DRB = fp8_k // 256     # fp8 DoubleRow blocks (256 k each)
    NT = nl // P           # n-tiles
    MT = tok // P          # m-tiles (token rows)
    nq = k // qs           # quant slices per m-tile
    nblk = qs // BLOCK     # 16-blocks per quant slice

    nc = bacc.Bacc("TRN2", target_bir_lowering=False, debug=False,
                   num_devices=N_CORES)

    x_d = nc.dram_tensor("x", [tok, k], dt.float32, kind="ExternalInput").ap()
    wt_d = nc.dram_tensor("wt", [k - fp8_k, nl], dt.bfloat16,
                          kind="ExternalInput").ap()
    b_d = nc.dram_tensor("bias", [nl, 1], dt.float32, kind="ExternalInput").ap()
    o_d = nc.dram_tensor("outT", [nl, tok], dt.float32, kind="ExternalOutput").ap()
    xq_d = nc.dram_tensor("xdeq", [tok, k - fp8_k], dt.bfloat16,
                          kind="ExternalOutput" if debug_xdeq else "Internal").ap()
    if fp8_k:
        # fp8 weights, DoubleRow-packed: row-slab b holds w8[p, i, n] with
        # k = b*256 + 2p + i
        w8_d = nc.dram_tensor("w8", [DRB * P, 2 * nl], dt.float8e4,
                              kind="ExternalInput").ap()
        xq8_d = nc.dram_tensor("xdeq8", [tok, fp8_k], dt.float8e4,
                               kind="Internal").ap()

    with tile.TileContext(nc) as tc:
        with (
            tc.tile_pool(name="const", bufs=1) as constp,
            tc.tile_pool(name="wres", bufs=1) as wres,
            tc.tile_pool(name="xin", bufs=3) as xin,
            tc.tile_pool(name="scal", bufs=2) as scal,
            tc.tile_pool(name="q2p", bufs=2) as q2p,
            tc.tile_pool(name="q8p", bufs=2) as q8p,
            tc.tile_pool(name="shp", bufs=2) as shp,
            tc.tile_pool(name="xtp", bufs=xt_slots) as xtp,
            tc.tile_pool(name="outp", bufs=2) as outp,
            tc.tile_pool(name="psum", bufs=4, space="PSUM") as psump,
        ):
            # ---- constants ----
            emask = constp.tile([P, 1], dt.float32, tag="emask")
            nc.vector._memset_packed(emask[:], 0x7F800000)
            bias_t = constp.tile([P, NT], dt.float32, tag="bias")
            for n in range(NT):
                nc.sync.dma_start(bias_t[:, n:n + 1], b_d[n * P:(n + 1) * P, :])

            wt_tiles = []
            w8_tiles = []

            def quant_slice(m, d):
                    xsl = xin.tile([P, qs], dt.float32, tag="xsl")
                    nc.sync.dma_start(
                        xsl[:], x_d[m * P:(m + 1) * P, d * qs:(d + 1) * qs])
                    # per-16-block absmax -> e4m3 scale -> s/2 and 2/s
                    amax = scal.tile([P, nblk], dt.float32, tag="amax")
                    nc.vector.tensor_reduce(
                        amax[:], xsl[:].rearrange("p (b s) -> p b s", s=BLOCK),
                        axis=mybir.AxisListType.X, op=mybir.AluOpType.max,
                        apply_absolute_value=True)
                    s8 = scal.tile([P, nblk], dt.float8e4, tag="s8")
                    nc.vector.tensor_scalar(
                        out=s8[:], in0=amax[:], scalar1=1.0 / 6.0, scalar2=None,
                        op0=mybir.AluOpType.mult)
                    sh = scal.tile([P, nblk], dt.float32, tag="sh")
                    nc.vector.tensor_scalar(
                        out=sh[:], in0=s8[:], scalar1=FP8_MIN, scalar2=0.5,
                        op0=mybir.AluOpType.max, op1=mybir.AluOpType.mult)
                    r2 = scal.tile([P, nblk], dt.float32, tag="r2")
                    rs = scal.tile([P, nblk], dt.float32, tag="rs")
                    nc.vector.reciprocal_approx_accurate(r2[:], sh[:], rs[:])
                    # v2 = x * 2/s (in-place over the x tile), then
                    # q2 = 2*fp4(v2/2) via binade-magic round
                    nc.vector._custom_dve(
                        ops["MULB"], out=xsl[:], in0=xsl[:],
                        in1=r2[:].unsqueeze(2).to_broadcast((P, nblk, BLOCK)))
                    q2 = q2p.tile([P, qs], dt.bfloat16, tag="q2")
                    nc.vector._custom_dve(
                        ops["ROUND"], out=q2[:], in0=xsl[:],
                        s0=emask[:, :], s1=MAGIC / 2, imm2=MAGIC)
                    # s/2 expanded to bf16 (ACT)
                    shx = shp.tile([P, qs], dt.bfloat16, tag="shx")
                    nc.scalar.activation(
                        shx[:].rearrange("p (b s) -> p b s", s=BLOCK),
                        sh[:].unsqueeze(2).to_broadcast((P, nblk, BLOCK)),
                        mybir.ActivationFunctionType.Copy, bias=0.0, scale=1.0)
                    # xdeq = q2 * s/2; leading-fp8_k columns cast to e4m3
                    mul_eng = nc.gpsimd if mul_on_gpsimd else nc.vector
                    k0, k1 = d * qs, (d + 1) * qs
                    n8 = min(max(fp8_k - k0, 0), qs)  # fp8 cols in this slice
                    if n8:
                        q8 = q8p.tile([P, qs], dt.float8e4, tag="q8")
                        mul_eng.tensor_tensor(
                            out=q8[:, :n8], in0=q2[:, :n8], in1=shx[:, :n8],
                            op=mybir.AluOpType.mult)
                        nc.sync.dma_start(
                            xq8_d[m * P:(m + 1) * P, k0:k0 + n8], q8[:, :n8])
                    if n8 < qs:
                        mul_eng.tensor_tensor(
                            out=q2[:, n8:], in0=q2[:, n8:], in1=shx[:, n8:],
                            op=mybir.AluOpType.mult)
                        nc.sync.dma_start(
                            xq_d[m * P:(m + 1) * P,
                                 k0 + n8 - fp8_k:k1 - fp8_k], q2[:, n8:])

            def matmul_chunk(t0, ck):
                # transposed activations: fp8 k-pairs ride as u16 through the
                # xbar transpose; SBUF layout [p, 2t+i] with k = blk*256+2p+i.
                # Transposes alternate between the two HWDGE queues (SP/ACT).
                xt8s = []
                for b in range(DRB):
                    xt8 = xtp.tile([P, chunk], dt.bfloat16, tag="xt", name="xt")[:, :ck]
                    nc.sync.dma_start_transpose(
                        xt8, xq8_d[t0:t0 + ck, b * 256:(b + 1) * 256].bitcast(
                            dt.bfloat16))
                    xt8s.append(xt8)
                xts = []
                for kk in range(K16):
                    xt = xtp.tile([P, chunk], dt.bfloat16, tag="xt", name="xt")[:, :ck]
                    nc.sync.dma_start_transpose(
                        xt, xq_d[t0:t0 + ck, kk * P:(kk + 1) * P])
                    xts.append(xt)
                for n in range(NT):
                    ps = psump.tile([P, chunk], dt.float32, tag="ps", name="ps")[:, :ck]
                    nmm = DRB + K16
                    for b in range(DRB):
                        rhs8 = xt8s[b].bitcast(dt.float8e4).rearrange(
                            "p (t i) -> p i t", i=2)
                        lhs8 = w8_tiles[b][:].rearrange(
                            "p (i n) -> p i n", i=2)[:, :, n * P:(n + 1) * P]
                        nc.tensor.matmul(
                            ps, lhs8, rhs8, start=(b == 0), stop=False,
                            perf_mode=mybir.MatmulPerfMode.DoubleRow,
                            skip_group_check=True)
                    for kk in range(K16):
                        nc.tensor.matmul(
                            ps, wt_tiles[kk][:, n * P:(n + 1) * P], xts[kk],
                            start=(DRB == 0 and kk == 0), stop=(kk == K16 - 1),
                            skip_group_check=bool(DRB))
                    ob = outp.tile([P, chunk], dt.float32, tag="ob", name="ob")[:, :ck]
                    nc.scalar.activation(
                        ob, ps, mybir.ActivationFunctionType.Identity,
                        bias=bias_t[:, n:n + 1], scale=1.0)
                    nc.sync.dma_start(
                        o_d[n * P:(n + 1) * P, t0:t0 + ck], ob)

            # first chunk's quant is issued BEFORE the resident-weight loads so
            # its x DMA + DVE work overlaps the weight fetch; quant runs
            # d-major within each chunk so transposable columns finish early
            t0 = 0
            mdone = 0
            wt_loaded = False
            for ck in chunks:
                t0n = t0 + ck
                m_lo, m_hi = mdone, t0n // P
                for d in range(nq):
                    for m in range(m_lo, m_hi):
                        quant_slice(m, d)
                mdone = m_hi
                if not wt_loaded:
                    for b in range(DRB):
                        t = wres.tile([P, 2 * nl], dt.float8e4, tag=f"w8{b}")
                        nc.sync.dma_start(t[:], w8_d[b * P:(b + 1) * P, :])
                        w8_tiles.append(t)
                    for kk in range(K16):
                        t = wres.tile([P, nl], dt.bfloat16, tag=f"wt{kk}")
                        nc.sync.dma_start(t[:], wt_d[kk * P:(kk + 1) * P, :])
                        wt_tiles.append(t)
                    wt_loaded = True
                matmul_chunk(t0, ck)
                t0 = t0n

    nc.compile()
    _NC_CACHE[key] = nc
    return nc


def _prep_weights(weight_q, weight_scale, bias, fp8_k=FP8_K):
    """Host prepack: per-core transposed dequantized weights.
    Leading fp8_k columns as e4m3 in DoubleRow pair layout (k = b*256+2p+i),
    remainder as plain transposed bf16."""
    import ml_dtypes
    wq = np.asarray(weight_q, np.float32).reshape(OUT_F, K // BLOCK, BLOCK)
    ws = np.asarray(weight_scale, np.float32)[:, :, None]
    wdeq = (wq * ws).reshape(OUT_F, K)  # exact: <=6 significand bits
    wts, w8s, biases = [], [], []
    for c in range(N_CORES):
        sl = wdeq[c * NL:(c + 1) * NL]          # [NL, K]
        wts.append(np.ascontiguousarray(sl[:, fp8_k:].T).astype(
            ml_dtypes.bfloat16))
        if fp8_k:
            w8 = sl[:, :fp8_k].T.astype(ml_dtypes.float8_e4m3fn)  # [fp8_k, NL]
            w8s.append(np.ascontiguousarray(
                w8.reshape(fp8_k // 256, 128, 2, NL).reshape(
                    (fp8_k // 256) * 128, 2 * NL)))
        biases.append(np.ascontiguousarray(
            np.asarray(bias, np.float32)[c * NL:(c + 1) * NL].reshape(NL, 1)))
    return wts, w8s, biases


def make_in_maps(x, weight_q, weight_scale, bias):
    x2 = np.ascontiguousarray(np.asarray(x, np.float32).reshape(TOK, K))
    wts, w8s, biases = _prep_weights(weight_q, weight_scale, bias)
    in_maps = [{"x": x2, "wt": wts[c], "bias": biases[c]} for c in range(N_CORES)]
    if FP8_K:
        for c in range(N_CORES):
            in_maps[c]["w8"] = w8s[c]
    return in_maps


def kernel(x, weight_q, weight_scale, bias):
    from concourse.bass_utils import run_bass_kernel_spmd

    nc = build_nc()
    in_maps = make_in_maps(x, weight_q, weight_scale, bias)
    res = run_bass_kernel_spmd(nc, in_maps, list(range(N_CORES)))
    out = np.empty((TOK, OUT_F), np.float32)
    for c in range(N_CORES):
        out[:, c * NL:(c + 1) * NL] = res.results[c]["outT"].T
    return out.reshape(1, TOK, OUT_F)


if __name__ == "__main__":
    rng = np.random.default_rng(0)
    x = rng.normal(size=(1, TOK, K)).astype(np.float32)
    wq = rng.normal(size=(OUT_F, K)).astype(np.float32)
    ws = rng.random(size=(OUT_F, K // BLOCK)).astype(np.float32) + 0.1
    b = rng.normal(size=(OUT_F,)).astype(np.float32)
    out = kernel(x, wq, ws, b)
    print(out.shape, out.dtype)


# revision 49
# speedup vs baseline: 1.1311x; 1.0442x over previous
"""NVFP4 BlackwellLinear kernel for 8 Trainium2 NeuronCores.

Strategy (column-parallel, per sharding hint):
  - weight_q/weight_scale/bias are sharded along out_features (16384 -> 8 x 2048).
  - Weights are prepacked on host: w_deq = weight_q * weight_scale (exact in bf16,
    <= 6 significand bits), shipped pre-transposed as wt[K, N_loc] bf16.
  - x is replicated; each core quantizes the full activation tensor on-device,
    then does the bf16 matmul out^T = w_deq @ x_deq^T with bias fused into the
    PSUM->SBUF eviction. Host transposes/concats the per-core out^T slices.

The entire fp4 round-to-nearest is ONE custom DVE op (QFP4) using a
binade-dependent magic number:
  v2 = x * r2                      (r2 = 2/s per 16-block, broadcast)
  b  = v2 & 0x7F800000             (unsigned binade; AND with +inf imm)
  G  = max(b * M/2, M)             (M = 1.5*2^23; G in {M, 2M, 4M, ...})
  q2 = (v2 + G) - G                == 2*fp4(x/s) exactly for |v2| <= 12.8
Rounding to multiples of 2^t where t = max(0, exp(v2)-2) lands exactly on the
fp4 e2m1 grid {0,1,2,3,4,6,8,12} (x2). Valid while |x/s| <= 7 (guaranteed:
s >= (amax/6)*(15/16) for normal-range scales; blocks with amax < ~0.018
could violate via e4m3-subnormal scale error, never seen for N(0,1) data).
Then xdeq = q2 * (s/2) in bf16 (exact, <= 6 significand bits).
"""

import os
import numpy as np

TOK = 4096
K = 4096
OUT_F = 16384
N_CORES = 8
NL = OUT_F // N_CORES  # 2048
P = 128
BLOCK = 16

# tunables
CHUNK = 512          # max token chunk for the matmul phase (rhs free dim)
CHUNKS = (512,) * 8
QS = 2048            # quant compute slice (free elems) == x DMA granularity
XT_SLOTS = 44        # xT tile slots ([P, CHUNK] bf16 each)
MUL_ON_GPSIMD = False
LDW_OPT = False      # walrus LDW opt is incompatible with bass ldweights
FP8_K = 2560         # leading k-extent computed in fp8 e4m3 DoubleRow (mult of 256)

MAGIC = 12582912.0   # 1.5 * 2^23
FP8_MIN = 2.0 ** -9

_REGISTERED = {}


def _register_ops():
    """Register the custom DVE ops (idempotent). shas computed dynamically."""
    if _REGISTERED:
        return _REGISTERED
    import concourse.dve_ops as dve_ops
    from concourse.dve_ops import DveOp
    from concourse.dve_spec import (
        Spec, Src0, Src1, C0, C1, C2, lower, AluOp, Bin, maxx, _has_src1,
    )
    from concourse.dve_uop import DveOpSpec

    def ref_mulb(in0, in1, s0, s1, imm2):
        a = np.asarray(in0, np.float32)
        b = np.asarray(in1, np.float32).reshape(a.shape)
        return (a * b).astype(np.float32)

    spec_mulb = Spec(body=Src0 * Src1, reference=ref_mulb)

    def ref_round(in0, in1, s0, s1, imm2):
        v2 = np.asarray(in0, np.float32)
        b = (v2.view(np.uint32) & np.uint32(0x7F800000)).view(np.float32)
        G = np.maximum((b * np.float32(s1)).astype(np.float32),
                       np.float32(imm2))
        h = (v2 + G).astype(np.float32)
        return (h - G).astype(np.float32)

    # C0 = +inf mask AP (0x7F800000 per-partition), C1 = MAGIC/2, C2 = MAGIC
    b = Bin(AluOp.BITWISE_AND, Src0, C0)
    G = maxx(b * C1, C2)
    spec_round = Spec(body=(Src0 + G) - G, reference=ref_round)

    def mk(name, spec):
        shas = {}
        for ver in ("v3", "v4"):
            uops = lower(spec, ver=ver)
            row = dve_ops._CUSTOM_DVE_ROW_BASE + len(dve_ops.OPS)
            dos = DveOpSpec(name=name, opcode=row, uops=uops, rd1_en=_has_src1(spec))
            shas[ver] = dos.sha(ver)
        op = DveOp(name, spec, subdim=False, uops_sha=shas)
        dve_ops.OPS.append(op)
        dve_ops.CUSTOM_DVE_SPECS[name] = spec
        dve_ops._SUB_OPCODE_FOR_NAME[name] = dve_ops._CUSTOM_DVE_ROW_BASE + len(dve_ops.OPS) - 1
        return op

    _REGISTERED["MULB"] = mk("NVFP4_MULBCAST_ANT", spec_mulb)
    _REGISTERED["ROUND"] = mk("NVFP4_MAGICROUND_ANT", spec_round)
    return _REGISTERED


_NC_CACHE = {}


_LDW_ENABLE = [False]


def _patch_ldw_opt():
    """Flip walrus --enable-ldw-opt to true (hidden LDWEIGHTS via weight
    preload). The stock command line disables it; measured cost is ~43ns of
    exposed weight-load per 512-col matmul without it."""
    _LDW_ENABLE[0] = True
    from concourse import bass_utils as bu
    if getattr(bu, "_ldw_patched", False):
        return
    orig = bu.run_command

    def run_command_ldw(cmd, *a, **kw):
        if _LDW_ENABLE[0] and isinstance(cmd, list):
            cmd = ["--enable-ldw-opt=true" if c == "--enable-ldw-opt=false"
                   else c for c in cmd]
        return orig(cmd, *a, **kw)

    bu.run_command = run_command_ldw
    bu._ldw_patched = True


def build_nc(tok=TOK, k=K, nl=NL, chunk=CHUNK, qs=QS,
             xt_slots=XT_SLOTS, chunks=None, mul_on_gpsimd=MUL_ON_GPSIMD,
             ldw_opt=LDW_OPT, fp8_k=FP8_K, debug_xdeq=False):
    if ldw_opt:
        _patch_ldw_opt()
    assert fp8_k % 256 == 0 and 0 <= fp8_k < k
    if chunks is None:
        chunks = [c for c in CHUNKS if c <= tok]
        if sum(chunks) != tok:
            chunks = [chunk] * (tok // chunk)
    chunks = tuple(chunks)
    assert sum(chunks) == tok
    key = (tok, k, nl, chunk, qs, xt_slots, chunks, mul_on_gpsimd, ldw_opt,
           fp8_k, debug_xdeq)
    if key in _NC_CACHE:
        return _NC_CACHE[key]

    import concourse.bass as bass
    import concourse.mybir as mybir
    import concourse.tile as tile
    from concourse import bacc

    ops = _register_ops()
    dt = mybir.dt

    KT = k // P            # k-tiles (total)
    K16 = (k - fp8_k) // P  # bf16 k-tiles
    DRB = fp8_k // 256     # fp8 DoubleRow blocks (256 k each)
    NT = nl // P           # n-tiles
    MT = tok // P          # m-tiles (token rows)
    nq = k // qs           # quant slices per m-tile
    nblk = qs // BLOCK     # 16-blocks per quant slice

    nc = bacc.Bacc("TRN2", target_bir_lowering=False, debug=False,
                   num_devices=N_CORES)

    x_d = nc.dram_tensor("x", [tok, k], dt.float32, kind="ExternalInput").ap()
    wt_d = nc.dram_tensor("wt", [k - fp8_k, nl], dt.bfloat16,
                          kind="ExternalInput").ap()
    b_d = nc.dram_tensor("bias", [nl, 1], dt.float32, kind="ExternalInput").ap()
    o_d = nc.dram_tensor("outT", [nl, tok], dt.float32, kind="ExternalOutput").ap()
    xq_d = nc.dram_tensor("xdeq", [tok, k - fp8_k], dt.bfloat16,
                          kind="ExternalOutput" if debug_xdeq else "Internal").ap()
    if fp8_k:
        # fp8 weights, DoubleRow-packed: row-slab b holds w8[p, i, n] with
        # k = b*256 + 2p + i
        w8_d = nc.dram_tensor("w8", [DRB * P, 2 * nl], dt.float8e4,
                              kind="ExternalInput").ap()
        xq8_d = nc.dram_tensor("xdeq8", [tok, fp8_k], dt.float8e4,
                               kind="Internal").ap()

    with tile.TileContext(nc) as tc:
        with (
            tc.tile_pool(name="const", bufs=1) as constp,
            tc.tile_pool(name="wres", bufs=1) as wres,
            tc.tile_pool(name="xin", bufs=3) as xin,
            tc.tile_pool(name="scal", bufs=3) as scal,
            tc.tile_pool(name="q2p", bufs=3) as q2p,
            tc.tile_pool(name="q8p", bufs=2) as q8p,
            tc.tile_pool(name="shp", bufs=3) as shp,
            tc.tile_pool(name="xtp", bufs=xt_slots) as xtp,
            tc.tile_pool(name="outp", bufs=2) as outp,
            tc.tile_pool(name="psum", bufs=4, space="PSUM") as psump,
        ):
            # ---- constants ----
            emask = constp.tile([P, 1], dt.float32, tag="emask")
            nc.vector._memset_packed(emask[:], 0x7F800000)
            bias_t = constp.tile([P, NT], dt.float32, tag="bias")
            for n in range(NT):
                nc.sync.dma_start(bias_t[:, n:n + 1], b_d[n * P:(n + 1) * P, :])

            wt_tiles = []
            w8_tiles = []

            def quant_slice(m, d):
                    xsl = xin.tile([P, qs], dt.float32, tag="xsl")
                    nc.sync.dma_start(
                        xsl[:], x_d[m * P:(m + 1) * P, d * qs:(d + 1) * qs])
                    # per-16-block absmax -> e4m3 scale -> s/2 and 2/s
                    amax = scal.tile([P, nblk], dt.float32, tag="amax")
                    nc.vector.tensor_reduce(
                        amax[:], xsl[:].rearrange("p (b s) -> p b s", s=BLOCK),
                        axis=mybir.AxisListType.X, op=mybir.AluOpType.max,
                        apply_absolute_value=True)
                    s8 = scal.tile([P, nblk], dt.float8e4, tag="s8")
                    nc.vector.tensor_scalar(
                        out=s8[:], in0=amax[:], scalar1=1.0 / 6.0, scalar2=None,
                        op0=mybir.AluOpType.mult)
                    sh = scal.tile([P, nblk], dt.float32, tag="sh")
                    nc.vector.tensor_scalar(
                        out=sh[:], in0=s8[:], scalar1=FP8_MIN, scalar2=0.5,
                        op0=mybir.AluOpType.max, op1=mybir.AluOpType.mult)
                    r2 = scal.tile([P, nblk], dt.float32, tag="r2")
                    rs = scal.tile([P, nblk], dt.float32, tag="rs")
                    nc.vector.reciprocal_approx_accurate(r2[:], sh[:], rs[:])
                    # v2 = x * 2/s (in-place over the x tile), then
                    # q2 = 2*fp4(v2/2) via binade-magic round
                    nc.vector._custom_dve(
                        ops["MULB"], out=xsl[:], in0=xsl[:],
                        in1=r2[:].unsqueeze(2).to_broadcast((P, nblk, BLOCK)))
                    q2 = q2p.tile([P, qs], dt.bfloat16, tag="q2")
                    nc.vector._custom_dve(
                        ops["ROUND"], out=q2[:], in0=xsl[:],
                        s0=emask[:, :], s1=MAGIC / 2, imm2=MAGIC)
                    # s/2 expanded to bf16 (ACT)
                    shx = shp.tile([P, qs], dt.bfloat16, tag="shx")
                    nc.scalar.activation(
                        shx[:].rearrange("p (b s) -> p b s", s=BLOCK),
                        sh[:].unsqueeze(2).to_broadcast((P, nblk, BLOCK)),
                        mybir.ActivationFunctionType.Copy, bias=0.0, scale=1.0)
                    # xdeq = q2 * s/2; leading-fp8_k columns cast to e4m3
                    mul_eng = nc.gpsimd if mul_on_gpsimd else nc.vector
                    k0, k1 = d * qs, (d + 1) * qs
                    n8 = min(max(fp8_k - k0, 0), qs)  # fp8 cols in this slice
                    if n8:
                        q8 = q8p.tile([P, qs], dt.float8e4, tag="q8")
                        mul_eng.tensor_tensor(
                            out=q8[:, :n8], in0=q2[:, :n8], in1=shx[:, :n8],
                            op=mybir.AluOpType.mult)
                        nc.sync.dma_start(
                            xq8_d[m * P:(m + 1) * P, k0:k0 + n8], q8[:, :n8])
                    if n8 < qs:
                        mul_eng.tensor_tensor(
                            out=q2[:, n8:], in0=q2[:, n8:], in1=shx[:, n8:],
                            op=mybir.AluOpType.mult)
                        nc.sync.dma_start(
                            xq_d[m * P:(m + 1) * P,
                                 k0 + n8 - fp8_k:k1 - fp8_k], q2[:, n8:])

            def matmul_chunk(t0, ck):
                # transposed activations: fp8 k-pairs ride as u16 through the
                # xbar transpose; SBUF layout [p, 2t+i] with k = blk*256+2p+i.
                # Transposes alternate between the two HWDGE queues (SP/ACT).
                xt8s = []
                for b in range(DRB):
                    xt8 = xtp.tile([P, chunk], dt.bfloat16, tag="xt", name="xt")[:, :ck]
                    nc.sync.dma_start_transpose(
                        xt8, xq8_d[t0:t0 + ck, b * 256:(b + 1) * 256].bitcast(
                            dt.bfloat16))
                    xt8s.append(xt8)
                xts = []
                for kk in range(K16):
                    xt = xtp.tile([P, chunk], dt.bfloat16, tag="xt", name="xt")[:, :ck]
                    nc.sync.dma_start_transpose(
                        xt, xq_d[t0:t0 + ck, kk * P:(kk + 1) * P])
                    xts.append(xt)
                for n in range(NT):
                    ps = psump.tile([P, chunk], dt.float32, tag="ps", name="ps")[:, :ck]
                    nmm = DRB + K16
                    for b in range(DRB):
                        rhs8 = xt8s[b].bitcast(dt.float8e4).rearrange(
                            "p (t i) -> p i t", i=2)
                        lhs8 = w8_tiles[b][:].rearrange(
                            "p (i n) -> p i n", i=2)[:, :, n * P:(n + 1) * P]
                        nc.tensor.matmul(
                            ps, lhs8, rhs8, start=(b == 0), stop=False,
                            perf_mode=mybir.MatmulPerfMode.DoubleRow,
                            skip_group_check=True)
                    for kk in range(K16):
                        nc.tensor.matmul(
                            ps, wt_tiles[kk][:, n * P:(n + 1) * P], xts[kk],
                            start=(DRB == 0 and kk == 0), stop=(kk == K16 - 1),
                            skip_group_check=bool(DRB))
                    ob = outp.tile([P, chunk], dt.float32, tag="ob", name="ob")[:, :ck]
                    nc.scalar.activation(
                        ob, ps, mybir.ActivationFunctionType.Identity,
                        bias=bias_t[:, n:n + 1], scale=1.0)
                    # out-DMA rides the ACT HWDGE queue so next chunk's
                    # transposes on the sync queue aren't FIFO-gated behind it
                    nc.scalar.dma_start(
                        o_d[n * P:(n + 1) * P, t0:t0 + ck], ob)

            # first chunk's quant is issued BEFORE the resident-weight loads so
            # its x DMA + DVE work overlaps the weight fetch; quant runs
            # d-major within each chunk so transposable columns finish early
            t0 = 0
            mdone = 0
            wt_loaded = False
            for ck in chunks:
                t0n = t0 + ck
                m_lo, m_hi = mdone, t0n // P
                for d in range(nq):
                    for m in range(m_lo, m_hi):
                        quant_slice(m, d)
                mdone = m_hi
                if not wt_loaded:
                    for b in range(DRB):
                        t = wres.tile([P, 2 * nl], dt.float8e4, tag=f"w8{b}")
                        nc.sync.dma_start(t[:], w8_d[b * P:(b + 1) * P, :])
                        w8_tiles.append(t)
                    for kk in range(K16):
                        t = wres.tile([P, nl], dt.bfloat16, tag=f"wt{kk}")
                        nc.sync.dma_start(t[:], wt_d[kk * P:(kk + 1) * P, :])
                        wt_tiles.append(t)
                    wt_loaded = True
                matmul_chunk(t0, ck)
                t0 = t0n

    nc.compile()
    _NC_CACHE[key] = nc
    return nc


def _prep_weights(weight_q, weight_scale, bias, fp8_k=FP8_K):
    """Host prepack: per-core transposed dequantized weights.
    Leading fp8_k columns as e4m3 in DoubleRow pair layout (k = b*256+2p+i),
    remainder as plain transposed bf16."""
    import ml_dtypes
    wq = np.asarray(weight_q, np.float32).reshape(OUT_F, K // BLOCK, BLOCK)
    ws = np.asarray(weight_scale, np.float32)[:, :, None]
    wdeq = (wq * ws).reshape(OUT_F, K)  # exact: <=6 significand bits
    wts, w8s, biases = [], [], []
    for c in range(N_CORES):
        sl = wdeq[c * NL:(c + 1) * NL]          # [NL, K]
        wts.append(np.ascontiguousarray(sl[:, fp8_k:].T).astype(
            ml_dtypes.bfloat16))
        if fp8_k:
            w8 = sl[:, :fp8_k].T.astype(ml_dtypes.float8_e4m3fn)  # [fp8_k, NL]
            w8s.append(np.ascontiguousarray(
                w8.reshape(fp8_k // 256, 128, 2, NL).reshape(
                    (fp8_k // 256) * 128, 2 * NL)))
        biases.append(np.ascontiguousarray(
            np.asarray(bias, np.float32)[c * NL:(c + 1) * NL].reshape(NL, 1)))
    return wts, w8s, biases


def make_in_maps(x, weight_q, weight_scale, bias):
    x2 = np.ascontiguousarray(np.asarray(x, np.float32).reshape(TOK, K))
    wts, w8s, biases = _prep_weights(weight_q, weight_scale, bias)
    in_maps = [{"x": x2, "wt": wts[c], "bias": biases[c]} for c in range(N_CORES)]
    if FP8_K:
        for c in range(N_CORES):
            in_maps[c]["w8"] = w8s[c]
    return in_maps


def kernel(x, weight_q, weight_scale, bias):
    from concourse.bass_utils import run_bass_kernel_spmd

    nc = build_nc()
    in_maps = make_in_maps(x, weight_q, weight_scale, bias)
    res = run_bass_kernel_spmd(nc, in_maps, list(range(N_CORES)))
    out = np.empty((TOK, OUT_F), np.float32)
    for c in range(N_CORES):
        out[:, c * NL:(c + 1) * NL] = res.results[c]["outT"].T
    return out.reshape(1, TOK, OUT_F)


if __name__ == "__main__":
    rng = np.random.default_rng(0)
    x = rng.normal(size=(1, TOK, K)).astype(np.float32)
    wq = rng.normal(size=(OUT_F, K)).astype(np.float32)
    ws = rng.random(size=(OUT_F, K // BLOCK)).astype(np.float32) + 0.1
    b = rng.normal(size=(OUT_F,)).astype(np.float32)
    out = kernel(x, wq, ws, b)
    print(out.shape, out.dtype)
